# revision 25
# baseline (speedup 1.0000x reference)
"""Multi-head self-attention (B=4, S=2048, E=768, H=12, D=64) on 8 TRN2 cores.

Sharding: core c -> (batch b = c//2, head-group g = c%2 of 6 heads).
Each core computes its 6 heads' full attention plus its partial output
projection; the host sums the two partial projections per batch element
(tensor-parallel "all-reduce" done on gather).

Per-core pipeline (all matmul operands fp16, fp32 PSUM accumulation):
  xT  <- DMA-transpose of x            [E, S]     fp16
  qT/kT = W^T-style proj               [384, S]   fp16  (scale folded in W_q)
  v   = x @ W_v + b_v (ones-augmented) [S, 390]   fp16
  per head: logitsT = kT^T q  -> exp (ACT, fp16 psum) -> weightedT = v_aug^T exp
  normalize by the ones-column denominator, concat heads -> wT [384, S]
  out_partial = wT^T-chunks @ W_proj + b_proj/2   [S, E]  fp32
"""
import sys

sys.path.insert(0, "/opt/trn_rl_repo")

import numpy as np

import concourse.bass as bass
import concourse.tile as tile
from concourse import bacc, mybir
from concourse.bass_utils import run_bass_kernel_spmd

B, S, E = 4, 2048, 768
H, D = 12, 64
HG = 6                 # heads per core
FG = HG * D            # 384: per-core q/k/v feature dim
SCALE = D ** -0.5
F16 = mybir.dt.float16
F32 = mybir.dt.float32

N_CORES = 8
E_T = E // 128         # 6 E-chunks
F_T = FG // 128        # 3 f-tiles
S_T = S // 128         # 16 sequence tiles
SB = 512               # psum fp32 free block
S_B = S // SB          # 4


def _emit(nc, tc, ap, debug=False):
    ExpF = mybir.ActivationFunctionType.Exp
    persist_cm = tc.tile_pool(name="persist", bufs=1)
    persist = persist_cm.__enter__()

    # ---- persistent SBUF tensors ----
    xT = persist.tile([128, E_T, S], F16, tag="xT")
    qT = persist.tile([128, F_T, S], F16, tag="qT")
    kT = persist.tile([128, F_T, S], F16, tag="kT")
    v_sb = persist.tile([128, S_T, HG, D + 1], F16, tag="v")
    wT = persist.tile([128, F_T, S], F16, tag="wT")
    wn_tmp = persist.tile([64, S], F16, tag="wn_tmp")
    wq_sb = persist.tile([128, E_T, FG], F16, tag="wq")
    wk_sb = persist.tile([128, E_T, FG], F16, tag="wk")
    wv_sb = persist.tile([128, E_T, FG], F16, tag="wv")
    wp_sb = persist.tile([128, F_T, E], F16, tag="wp")
    bq_sb = persist.tile([128, F_T], F32, tag="bq")
    bk_sb = persist.tile([128, F_T], F32, tag="bk")
    bv_b = persist.tile([128, FG], F32, tag="bv_b")
    bp_b = persist.tile([128, E], F32, tag="bp_b")
    warm = persist.tile([128, 1], F32, tag="warm")

    # ---- input DMAs ----
    # all 6 DMA-transposes back-to-back first (interleaving plain DMAs with
    # DMA_TRANSPOSE forces xbar-mode serialization round-trips), then the
    # consolidated weight loads; broadcasts ride the SWDGE queue
    nc.sync.dma_start(out=wk_sb, in_=ap["wk"].rearrange("(e p) f -> p e f", p=128))
    nc.sync.dma_start(out=bk_sb, in_=ap["bk"].rearrange("(t p) -> p t", p=128))
    for e in range(E_T):
        nc.sync.dma_start_transpose(out=xT[:, e, :], in_=ap["x"][:, e * 128:(e + 1) * 128])
    nc.sync.dma_start(out=wq_sb, in_=ap["wq"].rearrange("(e p) f -> p e f", p=128))
    nc.sync.dma_start(out=bq_sb, in_=ap["bq"].rearrange("(t p) -> p t", p=128))
    nc.sync.dma_start(out=wv_sb, in_=ap["wv"].rearrange("(e p) f -> p e f", p=128))
    nc.sync.dma_start(out=wp_sb, in_=ap["wp"].rearrange("(f p) e -> p f e", p=128))
    # partition-broadcast biases
    bv_src = ap["bv"]
    nc.gpsimd.dma_start(
        out=bv_b,
        in_=bass.AP(tensor=bv_src.tensor, offset=bv_src.offset, ap=[[0, 128], [1, FG]]),
    )
    bp_src = ap["bp"]
    nc.gpsimd.dma_start(
        out=bp_b,
        in_=bass.AP(tensor=bp_src.tensor, offset=bp_src.offset, ap=[[0, 128], [1, E]]),
    )

    # ones column of v_aug (last col for every head)
    nc.vector.memset(v_sb[:, :, :, D:D + 1], 1.0)
    # prime the exp table set early
    nc.vector.memset(warm, 0.0)
    nc.scalar.activation(warm, warm, ExpF)

    # ---- phase 1: qT/kT (f-tile 0 first), v, then remaining f-tiles ----
    def emit_qk(ph1, f, tag, order=None):
        # sequential S-block accumulation: one live psum slot at a time so
        # banks release progressively instead of at phase-1 end
        if order is None:
            order = ((wq_sb, bq_sb, qT), (wk_sb, bk_sb, kT))
        for (w_sb, b_sb, dst) in order:
            for sb_i in range(S_B):
                ps = ph1.tile([128, SB], F32, tag=tag,
                              name=f"p1_{f}_{sb_i}_{dst.name}")
                for e in range(E_T):
                    nc.tensor.matmul(
                        ps,
                        w_sb[:, e, f * 128:(f + 1) * 128],
                        xT[:, e, sb_i * SB:(sb_i + 1) * SB],
                        start=(e == 0), stop=(e == E_T - 1),
                    )
                nc.vector.tensor_scalar_add(
                    out=dst[:, f, sb_i * SB:(sb_i + 1) * SB],
                    in0=ps,
                    scalar1=b_sb[:, f:f + 1],
                )

    # two separate 2-bank pools: pool A's banks (shared later by the wt
    # accumulator pool) are released when ft0+v finish, not at phase-1 end
    with (
        tc.tile_pool(name="ph1a", bufs=2, space="PSUM") as ph1a,
        tc.tile_pool(name="ph1b", bufs=2, space="PSUM") as ph1b,
    ):
        emit_qk(ph1a, 0, "p1a", order=((wk_sb, bk_sb, kT), (wq_sb, bq_sb, qT)))
        for st in range(S_T):
            vp = ph1a.tile([128, FG], F32, tag="p1a", name=f"vp_{st}")
            for e in range(E_T):
                nc.tensor.matmul(
                    vp,
                    xT[:, e, st * 128:(st + 1) * 128],
                    wv_sb[:, e, :],
                    start=(e == 0), stop=(e == E_T - 1),
                )
            vp3 = vp.rearrange("p (h d) -> p h d", h=HG)
            bv3 = bv_b.rearrange("p (h d) -> p h d", h=HG)
            nc.vector.tensor_add(v_sb[:, st, :, 0:D], vp3, bv3)
        for f in range(1, F_T):
            emit_qk(ph1b, f, "p1b")

    # ---- phase 2: attention per head, S in halves of 1024 ----
    SH = 1024
    # ---- phase 2: attention per head, S in halves of 1024 ----
    wt_cm = tc.tile_pool(name="wtp", bufs=2, space="PSUM")
    wt_pool = wt_cm.__enter__()
    osb_cm = tc.tile_pool(name="osb", bufs=4)
    osb_pool = osb_cm.__enter__()

    def emit_proj_part(fc, st_list):
        # partial projection for f-chunk fc over the given S-tiles, borrowing
        # wt-pool slots. fc 0 adds bias and writes DRAM; later chunks
        # accumulate into DRAM via SWDGE CCE-add.
        for st in st_list:
            pe_ = wt_pool.tile([128, 896], F32, tag="wt", name=f"prj_{fc}_{st}")
            for eb in range(2):
                nc.tensor.matmul(
                    pe_[:, eb * 512:eb * 512 + 384],
                    wT[:, fc, st * 128:(st + 1) * 128],
                    wp_sb[:, fc, eb * 384:(eb + 1) * 384],
                    start=True, stop=True,
                )
            osb = osb_pool.tile([128, E], F32, tag="osb", name=f"osb_{fc}_{st}")
            for eb in range(2):
                if fc == 0:
                    nc.vector.tensor_add(
                        osb[:, eb * 384:(eb + 1) * 384],
                        pe_[:, eb * 512:eb * 512 + 384],
                        bp_b[:, eb * 384:(eb + 1) * 384],
                    )
                else:
                    nc.vector.tensor_copy(
                        osb[:, eb * 384:(eb + 1) * 384],
                        pe_[:, eb * 512:eb * 512 + 384],
                    )
            if fc == 0:
                nc.sync.dma_start(out=ap["out"][st * 128:(st + 1) * 128, :], in_=osb)
            else:
                nc.gpsimd.dma_start(
                    out=ap["out"][st * 128:(st + 1) * 128, :], in_=osb,
                    accum_op=mybir.AluOpType.add,
                )
    with (
        tc.tile_pool(name="lgt", bufs=2, space="PSUM") as lgt_pool,
        tc.tile_pool(name="ex", bufs=6) as ex_pool,
        tc.tile_pool(name="nrm", bufs=2) as nrm_pool,
    ):
        for h in range(HG):
            ft, fp = h // 2, (h % 2) * 64
            even = h % 2 == 0
            dp = 64  # denominator partition row
            for sh in range(2):
                s0 = sh * SH
                wt = wt_pool.tile([128, SH], F32, tag="wt")  # [65 used]; 2 banks
                for t in range(S_T):
                    lgt = lgt_pool.tile([128, SH], F32, tag="lgt")  # 2 banks
                    kslice = kT[fp:fp + 64, ft, t * 128:(t + 1) * 128]
                    for q_i in range(2):
                        nc.tensor.matmul(
                            lgt[:, q_i * SB:(q_i + 1) * SB],
                            kslice,
                            qT[fp:fp + 64, ft, s0 + q_i * SB:s0 + (q_i + 1) * SB],
                            start=True, stop=True,
                        )
                    ex = ex_pool.tile([128, SH], F16, tag="ex")
                    nc.scalar.activation(ex, lgt, ExpF)
                    for q_i in range(2):
                        nc.tensor.matmul(
                            wt[0:65, q_i * SB:(q_i + 1) * SB],
                            v_sb[:, t, h, :],
                            ex[:, q_i * SB:(q_i + 1) * SB],
                            start=(t == 0), stop=(t == S_T - 1),
                        )
                # normalize: reshape the denom row to [128, 8] so the
                # reciprocal runs on all DVE lanes instead of one
                rec = nrm_pool.tile([128, SH], F32, tag="rec")
                nc.vector.tensor_copy(rec[dp:dp + 1, :], wt[dp:dp + 1, :])
                r8 = nrm_pool.tile([128, 8], F32, tag="r8")
                nc.sync.dma_start(
                    out=r8, in_=rec[dp:dp + 1, :].rearrange("p (a b) -> p a b", b=8)
                )
                r8r = nrm_pool.tile([128, 8], F32, tag="r8r")
                nc.vector.reciprocal(r8r, r8)
                # reciprocal row to partition 0 (partition_broadcast reads p0)
                nc.sync.dma_start(
                    out=rec[0:1, :].rearrange("p (a b) -> p a b", b=8), in_=r8r
                )
                recb = nrm_pool.tile([128, SH], F32, tag="recb")
                nc.gpsimd.partition_broadcast(recb, rec[0:1, :])
                if even:
                    nc.vector.tensor_mul(
                        wT[0:64, ft, s0:s0 + SH], wt[0:64, :], recb[0:64, :]
                    )
                else:
                    nc.vector.tensor_mul(
                        wn_tmp[0:64, s0:s0 + SH], wt[0:64, :], recb[0:64, :]
                    )
                    nc.sync.dma_start(
                        out=wT[64:128, ft, s0:s0 + SH], in_=wn_tmp[0:64, s0:s0 + SH]
                    )
                if h >= 2:
                    # 4 partial-projection tiles per sh-boundary: fc0 during
                    # heads 2-3, fc1 during heads 4-5
                    boundary = (h - 2) * 2 + sh
                    fc, part = boundary // 4, boundary % 4
                    emit_proj_part(fc, range(part * 4, part * 4 + 4))
        # keep the PE busy across the last-normalize gap so the HAM clock
        # gate stays at 2.4 GHz when the projection starts
        for wk_i in range(24):
            warm_mm = lgt_pool.tile([128, SH], F32, tag="lgt", name=f"warmmm_{wk_i}")
            for q_i in range(2):
                nc.tensor.matmul(
                    warm_mm[:, q_i * SB:(q_i + 1) * SB],
                    kT[0:64, 0, 0:128],
                    qT[0:64, 0, q_i * SB:(q_i + 1) * SB],
                    start=True, stop=True,
                )
    # ---- phase 3: remaining projection chunk (fc=2) ----
    emit_proj_part(2, range(S_T))
    wt_cm.__exit__(None, None, None)
    osb_cm.__exit__(None, None, None)

    if debug:
        nc.sync.dma_start(out=ap["dbg_qT"], in_=qT)
        nc.sync.dma_start(out=ap["dbg_kT"], in_=kT)
        nc.sync.dma_start(out=ap["dbg_v"], in_=v_sb)
        nc.sync.dma_start(out=ap["dbg_wT"], in_=wT)
    persist_cm.__exit__(None, None, None)


def build_nc(debug=False):
    nc = bacc.Bacc()
    ap = {
        "x": nc.dram_tensor("x", [S, E], F16, kind="ExternalInput").ap(),
        "wq": nc.dram_tensor("wq", [E, FG], F16, kind="ExternalInput").ap(),
        "wk": nc.dram_tensor("wk", [E, FG], F16, kind="ExternalInput").ap(),
        "wv": nc.dram_tensor("wv", [E, FG], F16, kind="ExternalInput").ap(),
        "bq": nc.dram_tensor("bq", [FG], F32, kind="ExternalInput").ap(),
        "bk": nc.dram_tensor("bk", [FG], F32, kind="ExternalInput").ap(),
        "bv": nc.dram_tensor("bv", [FG], F32, kind="ExternalInput").ap(),
        "wp": nc.dram_tensor("wp", [FG, E], F16, kind="ExternalInput").ap(),
        "bp": nc.dram_tensor("bp", [E], F32, kind="ExternalInput").ap(),
        "out": nc.dram_tensor("out", [S, E], F32, kind="ExternalOutput").ap(),
    }
    if debug:
        ap["dbg_qT"] = nc.dram_tensor("dbg_qT", [128, F_T, S], F16, kind="ExternalOutput").ap()
        ap["dbg_kT"] = nc.dram_tensor("dbg_kT", [128, F_T, S], F16, kind="ExternalOutput").ap()
        ap["dbg_v"] = nc.dram_tensor("dbg_v", [128, S_T, HG, D + 1], F16, kind="ExternalOutput").ap()
        ap["dbg_wT"] = nc.dram_tensor("dbg_wT", [128, F_T, S], F16, kind="ExternalOutput").ap()
    with tile.TileContext(nc) as tc:
        _emit(nc, tc, ap, debug=debug)
    nc.compile()
    return nc


def make_in_maps(x, W_qkv, b_qkv, W_proj, b_proj):
    """Per-core input dicts. Core c -> (batch c//2, head-group c%2)."""
    x = np.asarray(x, np.float32)
    W = np.asarray(W_qkv, np.float32).reshape(E, 3, H, D)
    bqkv = np.asarray(b_qkv, np.float32).reshape(3, H, D)
    Wp = np.asarray(W_proj, np.float32)
    bp = np.asarray(b_proj, np.float32)
    maps = []
    for c in range(N_CORES):
        b, g = c // 2, c % 2
        hs = slice(g * HG, (g + 1) * HG)
        maps.append({
            "x": x[b].astype(np.float16),
            "wq": (W[:, 0, hs, :].reshape(E, FG) * SCALE).astype(np.float16),
            "wk": W[:, 1, hs, :].reshape(E, FG).astype(np.float16),
            "wv": W[:, 2, hs, :].reshape(E, FG).astype(np.float16),
            "bq": (bqkv[0, hs].reshape(FG) * SCALE).astype(np.float32),
            "bk": bqkv[1, hs].reshape(FG).astype(np.float32),
            "bv": bqkv[2, hs].reshape(FG).astype(np.float32),
            "wp": Wp[g * FG:(g + 1) * FG, :].astype(np.float16),
            "bp": (bp / 2.0).astype(np.float32),
        })
    return maps


_NC = None


def kernel(x, W_qkv, b_qkv, W_proj, b_proj):
    global _NC
    if _NC is None:
        _NC = build_nc()
    maps = make_in_maps(x, W_qkv, b_qkv, W_proj, b_proj)
    res = run_bass_kernel_spmd(_NC, maps, core_ids=list(range(N_CORES)))
    out = np.empty((B, S, E), np.float32)
    for b in range(B):
        out[b] = res.results[2 * b]["out"] + res.results[2 * b + 1]["out"]
    return out


# revision 26
# speedup vs baseline: 1.0600x; 1.0600x over previous
"""Multi-head self-attention (B=4, S=2048, E=768, H=12, D=64) on 8 TRN2 cores.

Sharding: core c -> (batch b = c//2, head-group g = c%2 of 6 heads).
Each core computes its 6 heads' full attention plus its partial output
projection; the host sums the two partial projections per batch element
(tensor-parallel "all-reduce" done on gather).

Per-core pipeline (all matmul operands fp16, fp32 PSUM accumulation):
  xT  <- DMA-transpose of x            [E, S]     fp16
  qT/kT = W^T-style proj               [384, S]   fp16  (scale folded in W_q)
  v   = x @ W_v + b_v (ones-augmented) [S, 390]   fp16
  per head: logitsT = kT^T q  -> exp (ACT, fp16 psum) -> weightedT = v_aug^T exp
  normalize by the ones-column denominator, concat heads -> wT [384, S]
  out_partial = wT^T-chunks @ W_proj + b_proj/2   [S, E]  fp32
"""
import sys

sys.path.insert(0, "/opt/trn_rl_repo")

import numpy as np

import concourse.bass as bass
import concourse.tile as tile
from concourse import bacc, mybir
from concourse.bass_utils import run_bass_kernel_spmd

B, S, E = 4, 2048, 768
H, D = 12, 64
HG = 6                 # heads per core
FG = HG * D            # 384: per-core q/k/v feature dim
SCALE = D ** -0.5
F16 = mybir.dt.float16
F32 = mybir.dt.float32

N_CORES = 8
E_T = E // 128         # 6 E-chunks
F_T = FG // 128        # 3 f-tiles
S_T = S // 128         # 16 sequence tiles
SB = 512               # psum fp32 free block
S_B = S // SB          # 4


def _emit(nc, tc, ap, debug=False):
    ExpF = mybir.ActivationFunctionType.Exp
    persist_cm = tc.tile_pool(name="persist", bufs=1)
    persist = persist_cm.__enter__()

    # ---- persistent SBUF tensors ----
    xT = persist.tile([128, E_T, S], F16, tag="xT")
    qT = persist.tile([128, F_T, S], F16, tag="qT")
    kT = persist.tile([128, F_T, S], F16, tag="kT")
    v_sb = persist.tile([128, S_T, HG, D + 1], F16, tag="v")
    wT = persist.tile([128, F_T, S], F16, tag="wT")
    wn_tmp = persist.tile([64, S], F16, tag="wn_tmp")
    wq_sb = persist.tile([128, E_T, FG], F16, tag="wq")
    wk_sb = persist.tile([128, E_T, FG], F16, tag="wk")
    wv_sb = persist.tile([128, E_T, FG], F16, tag="wv")
    wp_sb = persist.tile([128, F_T, E], F16, tag="wp")
    bq_sb = persist.tile([128, F_T], F32, tag="bq")
    bk_sb = persist.tile([128, F_T], F32, tag="bk")
    bv_b = persist.tile([128, FG], F32, tag="bv_b")
    bp_b = persist.tile([128, E], F32, tag="bp_b")
    warm = persist.tile([128, 1], F32, tag="warm")

    # ---- input DMAs ----
    # all 6 DMA-transposes back-to-back first (interleaving plain DMAs with
    # DMA_TRANSPOSE forces xbar-mode serialization round-trips), then the
    # consolidated weight loads; broadcasts ride the SWDGE queue
    nc.sync.dma_start(out=wk_sb, in_=ap["wk"].rearrange("(e p) f -> p e f", p=128))
    nc.sync.dma_start(out=bk_sb, in_=ap["bk"].rearrange("(t p) -> p t", p=128))
    for e in range(E_T):
        nc.sync.dma_start_transpose(out=xT[:, e, :], in_=ap["x"][:, e * 128:(e + 1) * 128])
    nc.sync.dma_start(out=wq_sb, in_=ap["wq"].rearrange("(e p) f -> p e f", p=128))
    nc.sync.dma_start(out=bq_sb, in_=ap["bq"].rearrange("(t p) -> p t", p=128))
    nc.sync.dma_start(out=wv_sb, in_=ap["wv"].rearrange("(e p) f -> p e f", p=128))
    nc.sync.dma_start(out=wp_sb, in_=ap["wp"].rearrange("(f p) e -> p f e", p=128))
    # partition-broadcast biases
    bv_src = ap["bv"]
    nc.gpsimd.dma_start(
        out=bv_b,
        in_=bass.AP(tensor=bv_src.tensor, offset=bv_src.offset, ap=[[0, 128], [1, FG]]),
    )
    bp_src = ap["bp"]
    nc.gpsimd.dma_start(
        out=bp_b,
        in_=bass.AP(tensor=bp_src.tensor, offset=bp_src.offset, ap=[[0, 128], [1, E]]),
    )

    # ones column of v_aug (last col for every head)
    nc.vector.memset(v_sb[:, :, :, D:D + 1], 1.0)
    # prime the exp table set early
    nc.vector.memset(warm, 0.0)
    nc.scalar.activation(warm, warm, ExpF)

    # ---- phase 1: qT/kT (f-tile 0 first), v, then remaining f-tiles ----
    def emit_qk(ph1, f, tag, order=None):
        # sequential S-block accumulation: one live psum slot at a time so
        # banks release progressively instead of at phase-1 end
        if order is None:
            order = ((wq_sb, bq_sb, qT), (wk_sb, bk_sb, kT))
        for (w_sb, b_sb, dst) in order:
            for sb_i in range(S_B):
                ps = ph1.tile([128, SB], F32, tag=tag,
                              name=f"p1_{f}_{sb_i}_{dst.name}")
                for e in range(E_T):
                    nc.tensor.matmul(
                        ps,
                        w_sb[:, e, f * 128:(f + 1) * 128],
                        xT[:, e, sb_i * SB:(sb_i + 1) * SB],
                        start=(e == 0), stop=(e == E_T - 1),
                    )
                nc.vector.tensor_scalar_add(
                    out=dst[:, f, sb_i * SB:(sb_i + 1) * SB],
                    in0=ps,
                    scalar1=b_sb[:, f:f + 1],
                )

    # two separate 2-bank pools: pool A's banks (shared later by the wt
    # accumulator pool) are released when ft0+v finish, not at phase-1 end
    with (
        tc.tile_pool(name="ph1a", bufs=2, space="PSUM") as ph1a,
        tc.tile_pool(name="ph1b", bufs=2, space="PSUM") as ph1b,
    ):
        emit_qk(ph1a, 0, "p1a", order=((wk_sb, bk_sb, kT), (wq_sb, bq_sb, qT)))
        for st in range(S_T):
            vp = ph1a.tile([128, FG], F32, tag="p1a", name=f"vp_{st}")
            for e in range(E_T):
                nc.tensor.matmul(
                    vp,
                    xT[:, e, st * 128:(st + 1) * 128],
                    wv_sb[:, e, :],
                    start=(e == 0), stop=(e == E_T - 1),
                )
            vp3 = vp.rearrange("p (h d) -> p h d", h=HG)
            bv3 = bv_b.rearrange("p (h d) -> p h d", h=HG)
            nc.vector.tensor_add(v_sb[:, st, :, 0:D], vp3, bv3)
        for f in range(1, F_T):
            emit_qk(ph1b, f, "p1b")

    # ---- phase 2: attention per head, S in halves of 1024 ----
    SH = 1024
    # ---- phase 2: attention per head, S in halves of 1024 ----
    wt_cm = tc.tile_pool(name="wtp", bufs=2, space="PSUM")
    wt_pool = wt_cm.__enter__()
    with (
        tc.tile_pool(name="lgt", bufs=2, space="PSUM") as lgt_pool,
        tc.tile_pool(name="ex", bufs=6) as ex_pool,
        tc.tile_pool(name="nrm", bufs=2) as nrm_pool,
    ):
        for h in range(HG):
            ft, fp = h // 2, (h % 2) * 64
            even = h % 2 == 0
            dp = 64  # denominator partition row
            for sh in range(2):
                s0 = sh * SH
                wt = wt_pool.tile([128, SH], F32, tag="wt")  # [65 used]; 2 banks
                for t in range(S_T):
                    lgt = lgt_pool.tile([128, SH], F32, tag="lgt")  # 2 banks
                    kslice = kT[fp:fp + 64, ft, t * 128:(t + 1) * 128]
                    for q_i in range(2):
                        nc.tensor.matmul(
                            lgt[:, q_i * SB:(q_i + 1) * SB],
                            kslice,
                            qT[fp:fp + 64, ft, s0 + q_i * SB:s0 + (q_i + 1) * SB],
                            start=True, stop=True,
                        )
                    ex = ex_pool.tile([128, SH], F16, tag="ex")
                    nc.scalar.activation(ex, lgt, ExpF)
                    for q_i in range(2):
                        nc.tensor.matmul(
                            wt[0:65, q_i * SB:(q_i + 1) * SB],
                            v_sb[:, t, h, :],
                            ex[:, q_i * SB:(q_i + 1) * SB],
                            start=(t == 0), stop=(t == S_T - 1),
                        )
                # normalize: reshape the denom row to [128, 8] so the
                # reciprocal runs on all DVE lanes instead of one
                rec = nrm_pool.tile([128, SH], F32, tag="rec")
                nc.vector.tensor_copy(rec[dp:dp + 1, :], wt[dp:dp + 1, :])
                r8 = nrm_pool.tile([128, 8], F32, tag="r8")
                nc.sync.dma_start(
                    out=r8, in_=rec[dp:dp + 1, :].rearrange("p (a b) -> p a b", b=8)
                )
                r8r = nrm_pool.tile([128, 8], F32, tag="r8r")
                nc.vector.reciprocal(r8r, r8)
                # reciprocal row to partition 0 (partition_broadcast reads p0)
                nc.sync.dma_start(
                    out=rec[0:1, :].rearrange("p (a b) -> p a b", b=8), in_=r8r
                )
                recb = nrm_pool.tile([128, SH], F32, tag="recb")
                nc.gpsimd.partition_broadcast(recb, rec[0:1, :])
                if even:
                    nc.vector.tensor_mul(
                        wT[0:64, ft, s0:s0 + SH], wt[0:64, :], recb[0:64, :]
                    )
                else:
                    nc.vector.tensor_mul(
                        wn_tmp[0:64, s0:s0 + SH], wt[0:64, :], recb[0:64, :]
                    )
                    nc.sync.dma_start(
                        out=wT[64:128, ft, s0:s0 + SH], in_=wn_tmp[0:64, s0:s0 + SH]
                    )
        # keep the PE busy across the last-normalize gap so the HAM clock
        # gate stays at 2.4 GHz when the projection starts
        for wk_i in range(24):
            warm_mm = lgt_pool.tile([128, SH], F32, tag="lgt", name=f"warmmm_{wk_i}")
            for q_i in range(2):
                nc.tensor.matmul(
                    warm_mm[:, q_i * SB:(q_i + 1) * SB],
                    kT[0:64, 0, 0:128],
                    qT[0:64, 0, q_i * SB:(q_i + 1) * SB],
                    start=True, stop=True,
                )
    wt_cm.__exit__(None, None, None)

    # ---- phase 3: output projection (tail) ----
    with (
        tc.tile_pool(name="prj", bufs=4, space="PSUM") as prj_pool,
        tc.tile_pool(name="osb", bufs=3) as osb_pool,
    ):
        for st in range(S_T):
            pe_ = prj_pool.tile([128, 896], F32, tag="prj", name=f"prj_{st}")
            for fc in range(F_T):
                for eb in range(2):
                    nc.tensor.matmul(
                        pe_[:, eb * 512:eb * 512 + 384],
                        wT[:, fc, st * 128:(st + 1) * 128],
                        wp_sb[:, fc, eb * 384:(eb + 1) * 384],
                        start=(fc == 0), stop=(fc == F_T - 1),
                    )
            osb = osb_pool.tile([128, E], F32, tag="osb")
            for eb in range(2):
                nc.vector.tensor_add(
                    osb[:, eb * 384:(eb + 1) * 384],
                    pe_[:, eb * 512:eb * 512 + 384],
                    bp_b[:, eb * 384:(eb + 1) * 384],
                )
            eng = nc.sync if st % 2 == 0 else nc.gpsimd
            eng.dma_start(out=ap["out"][st * 128:(st + 1) * 128, :], in_=osb)

    if debug:
        nc.sync.dma_start(out=ap["dbg_qT"], in_=qT)
        nc.sync.dma_start(out=ap["dbg_kT"], in_=kT)
        nc.sync.dma_start(out=ap["dbg_v"], in_=v_sb)
        nc.sync.dma_start(out=ap["dbg_wT"], in_=wT)
    persist_cm.__exit__(None, None, None)


def build_nc(debug=False):
    nc = bacc.Bacc()
    ap = {
        "x": nc.dram_tensor("x", [S, E], F16, kind="ExternalInput").ap(),
        "wq": nc.dram_tensor("wq", [E, FG], F16, kind="ExternalInput").ap(),
        "wk": nc.dram_tensor("wk", [E, FG], F16, kind="ExternalInput").ap(),
        "wv": nc.dram_tensor("wv", [E, FG], F16, kind="ExternalInput").ap(),
        "bq": nc.dram_tensor("bq", [FG], F32, kind="ExternalInput").ap(),
        "bk": nc.dram_tensor("bk", [FG], F32, kind="ExternalInput").ap(),
        "bv": nc.dram_tensor("bv", [FG], F32, kind="ExternalInput").ap(),
        "wp": nc.dram_tensor("wp", [FG, E], F16, kind="ExternalInput").ap(),
        "bp": nc.dram_tensor("bp", [E], F32, kind="ExternalInput").ap(),
        "out": nc.dram_tensor("out", [S, E], F32, kind="ExternalOutput").ap(),
    }
    if debug:
        ap["dbg_qT"] = nc.dram_tensor("dbg_qT", [128, F_T, S], F16, kind="ExternalOutput").ap()
        ap["dbg_kT"] = nc.dram_tensor("dbg_kT", [128, F_T, S], F16, kind="ExternalOutput").ap()
        ap["dbg_v"] = nc.dram_tensor("dbg_v", [128, S_T, HG, D + 1], F16, kind="ExternalOutput").ap()
        ap["dbg_wT"] = nc.dram_tensor("dbg_wT", [128, F_T, S], F16, kind="ExternalOutput").ap()
    with tile.TileContext(nc) as tc:
        _emit(nc, tc, ap, debug=debug)
    nc.compile()
    return nc


def make_in_maps(x, W_qkv, b_qkv, W_proj, b_proj):
    """Per-core input dicts. Core c -> (batch c//2, head-group c%2)."""
    x = np.asarray(x, np.float32)
    W = np.asarray(W_qkv, np.float32).reshape(E, 3, H, D)
    bqkv = np.asarray(b_qkv, np.float32).reshape(3, H, D)
    Wp = np.asarray(W_proj, np.float32)
    bp = np.asarray(b_proj, np.float32)
    maps = []
    for c in range(N_CORES):
        b, g = c // 2, c % 2
        hs = slice(g * HG, (g + 1) * HG)
        maps.append({
            "x": x[b].astype(np.float16),
            "wq": (W[:, 0, hs, :].reshape(E, FG) * SCALE).astype(np.float16),
            "wk": W[:, 1, hs, :].reshape(E, FG).astype(np.float16),
            "wv": W[:, 2, hs, :].reshape(E, FG).astype(np.float16),
            "bq": (bqkv[0, hs].reshape(FG) * SCALE).astype(np.float32),
            "bk": bqkv[1, hs].reshape(FG).astype(np.float32),
            "bv": bqkv[2, hs].reshape(FG).astype(np.float32),
            "wp": Wp[g * FG:(g + 1) * FG, :].astype(np.float16),
            "bp": (bp / 2.0).astype(np.float32),
        })
    return maps


_NC = None


def kernel(x, W_qkv, b_qkv, W_proj, b_proj):
    global _NC
    if _NC is None:
        _NC = build_nc()
    maps = make_in_maps(x, W_qkv, b_qkv, W_proj, b_proj)
    res = run_bass_kernel_spmd(_NC, maps, core_ids=list(range(N_CORES)))
    out = np.empty((B, S, E), np.float32)
    for b in range(B):
        out[b] = res.results[2 * b]["out"] + res.results[2 * b + 1]["out"]
    return out
